# revision 1
# baseline (speedup 1.0000x reference)
"""Trainium2 Bass kernel for nn_Clash_net (clash energy over atom pairs).

Contract: kernel(**inputs) takes FULL (unsharded) numpy inputs as produced by
setup_inputs() and returns the FULL [6] float32 energies output.

Strategy (8 NeuronCores, SPMD):
  - Shard the atom-pairs dimension P across the 8 cores (contiguous split,
    padded with sentinel pairs whose clash contribution is exactly 0).
  - Replicate a packed per-atom table [x, y, z, r] (r = radii[atom_names])
    in DRAM on every core; per-pair endpoint records (16 B) are gathered
    on-device with GPSIMD indirect DMA, one [128,1]-offset call per 128
    records (the only offset form the DGE lowers correctly; measured
    ~0.37 us/call pipelined).
  - Per chunk: compute dist = sqrt(|c0-c1|^2 + eps), base = r0+r1-dist,
    then per class c: acc_c += mask_c * relu(base + tol_c)
    (relu via tensor_scalar add+max, masked sum via tensor_tensor_reduce).
  - Per-core partial [6] sums are returned; the host sums the 8 partials
    and scales by exp(weight[0]) (final unshard step).
"""

import sys

sys.path.insert(0, "/opt/trn_rl_repo")

import numpy as np

import concourse.bass as bass
import concourse.bacc as bacc
import concourse.mybir as mybir
import concourse.tile as tile
from concourse.bass_utils import run_bass_kernel_spmd

F32 = mybir.dt.float32
I32 = mybir.dt.int32
U8 = mybir.dt.uint8

N_CORES = 8
EPS = 1e-12

N_ATOMS = 100000
N_PAIRS = 4000000
N_CLASS = 6

PAIRS_PER_CORE = N_PAIRS // N_CORES  # 500000
CHUNK = 489
N_CHUNKS = 8
COLS = CHUNK * N_CHUNKS  # 3912
P_PAD = 128 * COLS  # 500736 >= 500000


def build_nc(p_pad, chunk, n_chunks, ntab, num_devices=N_CORES, repeat=1):
    """repeat>1 re-runs the whole pair loop (for delta-timing); output scales."""
    assert p_pad == 128 * chunk * n_chunks
    nc = bacc.Bacc(
        "TRN2", target_bir_lowering=False, debug=False, num_devices=num_devices
    )
    idx0 = nc.dram_tensor("idx0", [p_pad], I32, kind="ExternalInput")
    idx1 = nc.dram_tensor("idx1", [p_pad], I32, kind="ExternalInput")
    masks = nc.dram_tensor("masks", [N_CLASS, p_pad], U8, kind="ExternalInput")
    table = nc.dram_tensor("table", [ntab, 4], F32, kind="ExternalInput")
    toll = nc.dram_tensor("toll", [128, N_CLASS], F32, kind="ExternalInput")
    outp = nc.dram_tensor("out", [1, N_CLASS], F32, kind="ExternalOutput")

    with tile.TileContext(nc) as tc:
        with (
            tc.tile_pool(name="const", bufs=1) as cpool,
            tc.tile_pool(name="work", bufs=2) as wpool,
            tc.tile_pool(name="psum", bufs=1, space="PSUM") as ppool,
        ):
            n = chunk
            tolb = cpool.tile([128, N_CLASS], F32)
            nc.sync.dma_start(out=tolb[:], in_=toll[:])
            ones = cpool.tile([128, 1], F32)
            nc.vector.memset(ones[:], 1.0)
            epsb = cpool.tile([128, 1], F32)
            nc.vector.memset(epsb[:], EPS)
            acc = cpool.tile([128, N_CLASS], F32)
            nc.vector.memset(acc[:], 0.0)

            idx0_t = idx0[:].rearrange("(k p q) -> k p q", k=n_chunks, p=128)
            idx1_t = idx1[:].rearrange("(k p q) -> k p q", k=n_chunks, p=128)
            masks_t = masks[:].rearrange("c (k p q) -> c k p q", k=n_chunks, p=128)

            for k in [kk for _ in range(repeat) for kk in range(n_chunks)]:
                i0 = wpool.tile([128, n], I32, tag="i0")
                i1 = wpool.tile([128, n], I32, tag="i1")
                nc.sync.dma_start(out=i0[:], in_=idx0_t[k])
                nc.sync.dma_start(out=i1[:], in_=idx1_t[k])

                g0 = wpool.tile([128, n, 4], F32, tag="g0")
                g1 = wpool.tile([128, n, 4], F32, tag="g1")
                for j in range(n):
                    nc.gpsimd.indirect_dma_start(
                        out=g0[:, j, :],
                        out_offset=None,
                        in_=table[:],
                        in_offset=bass.IndirectOffsetOnAxis(ap=i0[:, j : j + 1], axis=0),
                    )
                for j in range(n):
                    nc.gpsimd.indirect_dma_start(
                        out=g1[:, j, :],
                        out_offset=None,
                        in_=table[:],
                        in_offset=bass.IndirectOffsetOnAxis(ap=i1[:, j : j + 1], axis=0),
                    )

                dx = wpool.tile([128, n], F32, tag="dx")
                dy = wpool.tile([128, n], F32, tag="dy")
                dz = wpool.tile([128, n], F32, tag="dz")
                rs = wpool.tile([128, n], F32, tag="rs")
                nc.vector.tensor_sub(out=dx[:], in0=g0[:, :, 0], in1=g1[:, :, 0])
                nc.vector.tensor_sub(out=dy[:], in0=g0[:, :, 1], in1=g1[:, :, 1])
                nc.vector.tensor_sub(out=dz[:], in0=g0[:, :, 2], in1=g1[:, :, 2])
                nc.vector.tensor_add(out=rs[:], in0=g0[:, :, 3], in1=g1[:, :, 3])

                ss = wpool.tile([128, n], F32, tag="ss")
                t2 = wpool.tile([128, n], F32, tag="t2")
                nc.vector.tensor_mul(out=ss[:], in0=dx[:], in1=dx[:])
                nc.vector.tensor_mul(out=t2[:], in0=dy[:], in1=dy[:])
                nc.vector.tensor_add(out=ss[:], in0=ss[:], in1=t2[:])
                nc.vector.tensor_mul(out=t2[:], in0=dz[:], in1=dz[:])
                nc.vector.tensor_add(out=ss[:], in0=ss[:], in1=t2[:])

                dist = wpool.tile([128, n], F32, tag="dist")
                nc.scalar.activation(
                    out=dist[:],
                    in_=ss[:],
                    func=mybir.ActivationFunctionType.Sqrt,
                    bias=epsb[:],
                )
                base = wpool.tile([128, n], F32, tag="base")
                nc.vector.tensor_sub(out=base[:], in0=rs[:], in1=dist[:])

                for c in range(N_CLASS):
                    mk = wpool.tile([128, n], U8, tag=f"mk{c}")
                    nc.sync.dma_start(out=mk[:], in_=masks_t[c, k])
                    rc = wpool.tile([128, n], F32, tag="rc")
                    nc.vector.tensor_scalar(
                        out=rc[:],
                        in0=base[:],
                        scalar1=tolb[:, c : c + 1],
                        scalar2=0.0,
                        op0=mybir.AluOpType.add,
                        op1=mybir.AluOpType.max,
                    )
                    scr = wpool.tile([128, n], F32, tag="scr")
                    nc.vector.tensor_tensor(
                        out=scr[:], in0=rc[:], in1=mk[:], op=mybir.AluOpType.mult
                    )
                    red = wpool.tile([128, 1], F32, tag="red")
                    nc.vector.tensor_reduce(
                        out=red[:],
                        in_=scr[:],
                        axis=mybir.AxisListType.X,
                        op=mybir.AluOpType.add,
                    )
                    nc.vector.tensor_add(
                        out=acc[:, c : c + 1], in0=acc[:, c : c + 1], in1=red[:]
                    )

            psum = ppool.tile([1, N_CLASS], F32, space="PSUM")
            nc.tensor.matmul(
                out=psum[:], lhsT=ones[:], rhs=acc[:], start=True, stop=True
            )
            out6 = cpool.tile([1, N_CLASS], F32)
            nc.vector.tensor_copy(out=out6[:], in_=psum[:])
            nc.sync.dma_start(out=outp[:], in_=out6[:])

    nc.compile()
    return nc


_NC_CACHE = {}


def _get_nc():
    key = (P_PAD, CHUNK, N_CHUNKS)
    if key not in _NC_CACHE:
        _NC_CACHE[key] = build_nc(P_PAD, CHUNK, N_CHUNKS, N_ATOMS + 2)
    return _NC_CACHE[key]


def _prep_inputs(coords, radii, tollerances, weight, atom_names, atom_pairs, clash_masks):
    """Host-side shard/layout prep. Returns (in_maps, exp_weight)."""
    coords = np.asarray(coords, dtype=np.float32)
    radii = np.asarray(radii, dtype=np.float32)
    tollerances = np.asarray(tollerances, dtype=np.float32)
    atom_names = np.asarray(atom_names)
    atom_pairs = np.asarray(atom_pairs)
    clash_masks = np.asarray(clash_masks)

    ntab = N_ATOMS + 2
    table = np.empty((ntab, 4), dtype=np.float32)
    table[:N_ATOMS, :3] = coords
    table[:N_ATOMS, 3] = radii[atom_names.astype(np.int64)]
    table[N_ATOMS] = (1e6, 1e6, 1e6, 0.0)
    table[N_ATOMS + 1] = (-1e6, -1e6, -1e6, 0.0)

    pairs32 = np.ascontiguousarray(atom_pairs.astype(np.int32))
    masks8 = np.ascontiguousarray(clash_masks).view(np.uint8)
    toll2d = np.ascontiguousarray(
        np.broadcast_to(tollerances.reshape(1, N_CLASS), (128, N_CLASS))
    )

    in_maps = []
    for c in range(N_CORES):
        lo, hi = c * PAIRS_PER_CORE, (c + 1) * PAIRS_PER_CORE
        i0 = np.full(P_PAD, N_ATOMS, dtype=np.int32)
        i1 = np.full(P_PAD, N_ATOMS + 1, dtype=np.int32)
        i0[:PAIRS_PER_CORE] = pairs32[lo:hi, 0]
        i1[:PAIRS_PER_CORE] = pairs32[lo:hi, 1]
        m = np.zeros((N_CLASS, P_PAD), dtype=np.uint8)
        m[:, :PAIRS_PER_CORE] = masks8[:, lo:hi]
        in_maps.append(
            {"idx0": i0, "idx1": i1, "masks": m, "table": table, "toll": toll2d}
        )
    return in_maps, float(np.exp(np.float64(np.asarray(weight).reshape(-1)[0])))


def kernel(coords, radii, tollerances, weight, atom_names, atom_pairs, clash_masks):
    nc = _get_nc()
    in_maps, wscale = _prep_inputs(
        coords, radii, tollerances, weight, atom_names, atom_pairs, clash_masks
    )
    res = run_bass_kernel_spmd(nc, in_maps, core_ids=list(range(N_CORES)))
    total = np.zeros(N_CLASS, dtype=np.float64)
    for c in range(N_CORES):
        total += res.results[c]["out"].reshape(N_CLASS).astype(np.float64)
    return (total * wscale).astype(np.float32)



# revision 3
# speedup vs baseline: 1.1541x; 1.1541x over previous
"""Trainium2 Bass kernel for nn_Clash_net (clash energy over atom pairs).

Contract: kernel(**inputs) takes FULL (unsharded) numpy inputs as produced by
setup_inputs() and returns the FULL [6] float32 energies output.

Strategy (8 NeuronCores, SPMD). The problem is gather-bound: each of the 4M
pairs needs two random 16 B records ([x,y,z,r]) from a 100K-atom table, and
the only correct dynamic-gather primitive on this hardware is the GPSIMD
indirect DMA with [128,1] offsets, measured at ~1.42 us per call (128 records)
regardless of row width.  The baseline needed 2*3912 = 7824 calls/core.

This kernel halves that:
  - Pairs are sorted by endpoint-0's atom id on the host and packed into
    batches of S=48 consecutive sorted pairs whose atoms span < W=16 table
    rows.  One [128,1]-offset indirect call then fetches 128 batches' windows
    (256 B rows, width is free), so endpoint-0 needs only 82 calls/core.
    The per-pair record is selected from its batch's 16-atom window on-device
    with a 16-way compare-select (DVE has ~8x slack under the Pool engine).
  - Endpoint-1 atoms are random w.r.t. the endpoint-0 sort (2D clustering is
    impossible at this pair density), so they use per-pair indirect gathers:
    3936 calls/core.  This is the irreducible bottleneck.
  - Per chunk: dist = sqrt(|c0-c1|^2 + eps), base = r0+r1-dist, then per
    class c: acc_c += mask_c * relu(base + tol_c); masks/idx/delta arrive in
    batch-grid layout prepared on the host.
  - Per-core partial [6] sums are returned; the host sums the 8 partials and
    scales by exp(weight[0]).
"""

import sys

sys.path.insert(0, "/opt/trn_rl_repo")

import numpy as np

import concourse.bass as bass
import concourse.bacc as bacc
import concourse.mybir as mybir
import concourse.tile as tile
from concourse.bass_utils import run_bass_kernel_spmd

F32 = mybir.dt.float32
I32 = mybir.dt.int32
U8 = mybir.dt.uint8

N_CORES = 8
EPS = 1e-12

N_ATOMS = 100000
N_PAIRS = 4000000
N_CLASS = 6

PAIRS_PER_CORE = N_PAIRS // N_CORES  # 500000

W = 16  # atoms per gather window (256 B rows)
S = 48  # pair slots per batch
NG = 82  # batch groups (128 batches each)
B_TOTAL = NG * 128  # 10496 batches >= ~10421 needed for uniform pairs
NCOLS = NG * S  # 3936 columns of 128 slots
NTAB = N_ATOMS + W + 2  # window reads stay in bounds

CHUNK_GROUPS = [16, 16, 16, 16, 16, 2]  # sums to NG
assert sum(CHUNK_GROUPS) == NG


def build_nc(num_devices=N_CORES, repeat=1):
    """repeat>1 re-runs the whole pair loop (for delta-timing); acc scales."""
    nc = bacc.Bacc(
        "TRN2", target_bir_lowering=False, debug=False, num_devices=num_devices
    )
    w0 = nc.dram_tensor("w0", [128, NG], I32, kind="ExternalInput")
    delta = nc.dram_tensor("delta", [128, NCOLS], U8, kind="ExternalInput")
    idx1 = nc.dram_tensor("idx1", [128, NCOLS], I32, kind="ExternalInput")
    masks = nc.dram_tensor("masks", [N_CLASS, 128, NCOLS], U8, kind="ExternalInput")
    table = nc.dram_tensor("table", [NTAB, 4], F32, kind="ExternalInput")
    toll = nc.dram_tensor("toll", [128, N_CLASS], F32, kind="ExternalInput")
    outp = nc.dram_tensor("out", [1, N_CLASS], F32, kind="ExternalOutput")

    with tile.TileContext(nc) as tc:
        with (
            tc.tile_pool(name="const", bufs=1) as cpool,
            tc.tile_pool(name="work", bufs=2) as wpool,
            tc.tile_pool(name="psum", bufs=1, space="PSUM") as ppool,
        ):
            tolb = cpool.tile([128, N_CLASS], F32)
            nc.sync.dma_start(out=tolb[:], in_=toll[:])
            ones = cpool.tile([128, 1], F32)
            nc.vector.memset(ones[:], 1.0)
            epsb = cpool.tile([128, 1], F32)
            nc.vector.memset(epsb[:], EPS)
            acc = cpool.tile([128, N_CLASS], F32)
            nc.vector.memset(acc[:], 0.0)
            w0t = cpool.tile([128, NG], I32)
            nc.sync.dma_start(out=w0t[:], in_=w0[:])

            masks_ap = masks[:]

            chunks = []
            g0 = 0
            for ng in CHUNK_GROUPS:
                chunks.append((g0, ng))
                g0 += ng

            for g0, ng in [ck for _ in range(repeat) for ck in chunks]:
                n = ng * S
                c0 = g0 * S

                idx1c = wpool.tile([128, n], I32, tag="idx1c")
                nc.sync.dma_start(out=idx1c[:], in_=idx1[:][:, c0 : c0 + n])
                dlt = wpool.tile([128, n], U8, tag="dlt")
                nc.sync.dma_start(out=dlt[:], in_=delta[:][:, c0 : c0 + n])

                graw = wpool.tile([128, ng, 4 * W], F32, tag="graw")
                for g in range(ng):
                    nc.gpsimd.indirect_dma_start(
                        out=graw[:, g, :],
                        out_offset=None,
                        in_=table[:],
                        in_offset=bass.IndirectOffsetOnAxis(
                            ap=w0t[:, g0 + g : g0 + g + 1], axis=0
                        ),
                    )
                g1 = wpool.tile([128, n, 4], F32, tag="g1")
                for j in range(n):
                    nc.gpsimd.indirect_dma_start(
                        out=g1[:, j, :],
                        out_offset=None,
                        in_=table[:],
                        in_offset=bass.IndirectOffsetOnAxis(
                            ap=idx1c[:, j : j + 1], axis=0
                        ),
                    )

                # 16-way select of endpoint-0 records from the batch windows
                xs = [
                    wpool.tile([128, ng, S], F32, tag=f"x{c}", name=f"x{c}")
                    for c in range(4)
                ]
                mt = wpool.tile([128, n], F32, tag="mt")
                tmp = wpool.tile([128, ng, S], F32, tag="tmp")
                for t in range(W):
                    nc.vector.tensor_scalar(
                        out=mt[:],
                        in0=dlt[:],
                        scalar1=float(t),
                        scalar2=None,
                        op0=mybir.AluOpType.is_equal,
                    )
                    mt3 = mt[:].rearrange("p (a b) -> p a b", a=ng)
                    for c in range(4):
                        src = graw[:, :, 4 * t + c : 4 * t + c + 1].to_broadcast(
                            [128, ng, S]
                        )
                        if t == 0:
                            nc.vector.tensor_tensor(
                                out=xs[c][:], in0=mt3, in1=src,
                                op=mybir.AluOpType.mult,
                            )
                        else:
                            nc.vector.tensor_tensor(
                                out=tmp[:], in0=mt3, in1=src,
                                op=mybir.AluOpType.mult,
                            )
                            nc.vector.tensor_add(
                                out=xs[c][:], in0=xs[c][:], in1=tmp[:]
                            )

                x0 = xs[0][:].rearrange("p a b -> p (a b)")
                y0 = xs[1][:].rearrange("p a b -> p (a b)")
                z0 = xs[2][:].rearrange("p a b -> p (a b)")
                r0 = xs[3][:].rearrange("p a b -> p (a b)")

                dx = wpool.tile([128, n], F32, tag="dx")
                dy = wpool.tile([128, n], F32, tag="dy")
                dz = wpool.tile([128, n], F32, tag="dz")
                rs = wpool.tile([128, n], F32, tag="rs")
                nc.vector.tensor_sub(out=dx[:], in0=x0, in1=g1[:, :, 0])
                nc.vector.tensor_sub(out=dy[:], in0=y0, in1=g1[:, :, 1])
                nc.vector.tensor_sub(out=dz[:], in0=z0, in1=g1[:, :, 2])
                nc.vector.tensor_add(out=rs[:], in0=r0, in1=g1[:, :, 3])

                ss = wpool.tile([128, n], F32, tag="ss")
                t2 = wpool.tile([128, n], F32, tag="t2")
                nc.vector.tensor_mul(out=ss[:], in0=dx[:], in1=dx[:])
                nc.vector.tensor_mul(out=t2[:], in0=dy[:], in1=dy[:])
                nc.vector.tensor_add(out=ss[:], in0=ss[:], in1=t2[:])
                nc.vector.tensor_mul(out=t2[:], in0=dz[:], in1=dz[:])
                nc.vector.tensor_add(out=ss[:], in0=ss[:], in1=t2[:])

                dist = wpool.tile([128, n], F32, tag="dist")
                nc.scalar.activation(
                    out=dist[:],
                    in_=ss[:],
                    func=mybir.ActivationFunctionType.Sqrt,
                    bias=epsb[:],
                )
                base = wpool.tile([128, n], F32, tag="base")
                nc.vector.tensor_sub(out=base[:], in0=rs[:], in1=dist[:])

                for c in range(N_CLASS):
                    mk = wpool.tile([128, n], U8, tag=f"mk{c}")
                    nc.sync.dma_start(out=mk[:], in_=masks_ap[c, :, c0 : c0 + n])
                    rc = wpool.tile([128, n], F32, tag="rc")
                    nc.vector.tensor_scalar(
                        out=rc[:],
                        in0=base[:],
                        scalar1=tolb[:, c : c + 1],
                        scalar2=0.0,
                        op0=mybir.AluOpType.add,
                        op1=mybir.AluOpType.max,
                    )
                    scr = wpool.tile([128, n], F32, tag="scr")
                    nc.vector.tensor_tensor(
                        out=scr[:], in0=rc[:], in1=mk[:], op=mybir.AluOpType.mult
                    )
                    red = wpool.tile([128, 1], F32, tag="red")
                    nc.vector.tensor_reduce(
                        out=red[:],
                        in_=scr[:],
                        axis=mybir.AxisListType.X,
                        op=mybir.AluOpType.add,
                    )
                    nc.vector.tensor_add(
                        out=acc[:, c : c + 1], in0=acc[:, c : c + 1], in1=red[:]
                    )

            psum = ppool.tile([1, N_CLASS], F32, space="PSUM")
            nc.tensor.matmul(
                out=psum[:], lhsT=ones[:], rhs=acc[:], start=True, stop=True
            )
            out6 = cpool.tile([1, N_CLASS], F32)
            nc.vector.tensor_copy(out=out6[:], in_=psum[:])
            nc.sync.dma_start(out=outp[:], in_=out6[:])

    nc.compile()
    return nc


_NC_CACHE = {}


def _get_nc(repeat=1):
    key = repeat
    if key not in _NC_CACHE:
        _NC_CACHE[key] = build_nc(repeat=repeat)
    return _NC_CACHE[key]


def _prep_core(pairs, masks8):
    """Batch one core's pairs. Returns per-core device arrays."""
    a0 = pairs[:, 0].astype(np.int64)
    a1 = pairs[:, 1].astype(np.int64)
    n = a0.shape[0]
    order = np.argsort(a0, kind="stable")
    a0s = a0[order]
    a1s = a1[order]

    # greedy batching: batch = up to S consecutive sorted pairs within a
    # W-atom window starting at the batch's first atom
    limit = np.searchsorted(a0s, a0s + W, side="left")
    starts = np.empty(B_TOTAL, dtype=np.int64)
    nb = 0
    cur = 0
    while cur < n:
        if nb >= B_TOTAL:
            raise RuntimeError(
                f"batch capacity exceeded ({nb} batches for {n} pairs); "
                f"pair distribution far from uniform"
            )
        starts[nb] = cur
        nb += 1
        cur = min(cur + S, int(limit[cur]))

    starts_r = starts[:nb]
    ends_r = np.append(starts_r[1:], n)

    spos = np.zeros((B_TOTAL, S), dtype=np.int64)
    valid = np.zeros((B_TOTAL, S), dtype=bool)
    ar = np.arange(S, dtype=np.int64)
    spos[:nb] = starts_r[:, None] + ar[None, :]
    valid[:nb] = spos[:nb] < ends_r[:, None]
    np.clip(spos, 0, n - 1, out=spos)

    w0_b = np.zeros(B_TOTAL, dtype=np.int32)
    w0_b[:nb] = a0s[starts_r].astype(np.int32)

    delta_b = (a0s[spos] - w0_b[:, None].astype(np.int64)).astype(np.uint8)
    delta_b[~valid] = 0
    idx1_b = a1s[spos].astype(np.int32)
    idx1_b[~valid] = 0

    pid = order[spos]  # original (core-local) pair index per slot
    masks_b = masks8[:, pid]  # [6, B_TOTAL, S]
    masks_b = masks_b * valid[None, :, :].astype(np.uint8)

    def to_dev(x):
        # batch b = g*128 + p  ->  partition p, cols [g*S, (g+1)*S)
        return np.ascontiguousarray(
            x.reshape(NG, 128, S).transpose(1, 0, 2).reshape(128, NCOLS)
        )

    return {
        "w0": np.ascontiguousarray(w0_b.reshape(NG, 128).T),
        "delta": to_dev(delta_b),
        "idx1": to_dev(idx1_b),
        "masks": np.ascontiguousarray(
            masks_b.reshape(N_CLASS, NG, 128, S)
            .transpose(0, 2, 1, 3)
            .reshape(N_CLASS, 128, NCOLS)
        ),
    }


def _prep_inputs(coords, radii, tollerances, weight, atom_names, atom_pairs, clash_masks):
    """Host-side shard/layout prep. Returns (in_maps, exp_weight)."""
    coords = np.asarray(coords, dtype=np.float32)
    radii = np.asarray(radii, dtype=np.float32)
    tollerances = np.asarray(tollerances, dtype=np.float32)
    atom_names = np.asarray(atom_names)
    atom_pairs = np.asarray(atom_pairs)
    clash_masks = np.asarray(clash_masks)

    table = np.zeros((NTAB, 4), dtype=np.float32)
    table[:N_ATOMS, :3] = coords
    table[:N_ATOMS, 3] = radii[atom_names.astype(np.int64)]

    pairs32 = np.ascontiguousarray(atom_pairs.astype(np.int32))
    masks8 = np.ascontiguousarray(clash_masks).view(np.uint8)
    toll2d = np.ascontiguousarray(
        np.broadcast_to(tollerances.reshape(1, N_CLASS), (128, N_CLASS))
    )

    in_maps = []
    for c in range(N_CORES):
        lo, hi = c * PAIRS_PER_CORE, (c + 1) * PAIRS_PER_CORE
        m = _prep_core(pairs32[lo:hi], masks8[:, lo:hi])
        m["table"] = table
        m["toll"] = toll2d
        in_maps.append(m)
    return in_maps, float(np.exp(np.float64(np.asarray(weight).reshape(-1)[0])))


def kernel(coords, radii, tollerances, weight, atom_names, atom_pairs, clash_masks):
    nc = _get_nc()
    in_maps, wscale = _prep_inputs(
        coords, radii, tollerances, weight, atom_names, atom_pairs, clash_masks
    )
    res = run_bass_kernel_spmd(nc, in_maps, core_ids=list(range(N_CORES)))
    total = np.zeros(N_CLASS, dtype=np.float64)
    for c in range(N_CORES):
        total += res.results[c]["out"].reshape(N_CLASS).astype(np.float64)
    return (total * wscale).astype(np.float32)
